# revision 1
# baseline (speedup 1.0000x reference)
"""GAT (3-layer) Trainium2 Bass kernel, 8-way node-sharded.

Self-contained: host preprocessing (graph partitioning, relabeling, edge
stream construction) + Bass/Tile kernel + gather/unshard.

Strategy:
  - Relabel nodes so core c owns new ids [c*NLOC, (c+1)*NLOC); blocks of 128
    dst nodes; per-block uniform chunk quotas (TE even-src + TO odd-src
    chunks of 128 edges each) with sentinel padding.
  - Per layer: each core computes its slice of the gather table
    (hf = h@W packed bf16 + attention scalars f32), AllGather replicates it.
  - Edge phase: dma_gather of per-edge rows (parity-split tables to fit int16
    indices), exp(lrelu(s_src+s_dst)) per edge, features scaled by per-head
    ee, one-hot matmul on PE accumulates per-dst sums + softmax denominators
    in PSUM.
  - Finalize per block: normalize, bias+BN affine, ELU, dense matmul for the
    next layer via PE transposes, next-layer attention scalars via W@A.
"""
import numpy as np
import ml_dtypes
from contextlib import ExitStack

import concourse.bacc as bacc
import concourse.bass as bass
import concourse.mybir as mybir
import concourse.tile as tile
from concourse.bass_utils import run_bass_kernel_spmd

P = 128
NCORES = 8
EPS_BN = 1e-5
NEG = -1e38
F32 = mybir.dt.float32
BF16 = mybir.dt.bfloat16
I16 = mybir.dt.int16
BF = ml_dtypes.bfloat16

# Full-problem constants (matches reference.py / spec.json)
N_FULL, E_FULL, F_IN, HID, HEADS, OUTD = 50000, 800000, 128, 64, 4, 2


# ----------------------------------------------------------------------------
# Host preprocessing
# ----------------------------------------------------------------------------

def preprocess(edge_index, n_nodes, nblk, te, to):
    """Graph partitioning + edge stream construction.

    Returns (new_id [n_nodes], plans per core). All integer index work only.
    """
    T = te + to
    nloc = nblk * P
    cap_e, cap_o = te * P, to * P
    src = np.concatenate([edge_index[0].astype(np.int64), np.arange(n_nodes)])
    dst = np.concatenate([edge_index[1].astype(np.int64), np.arange(n_nodes)])
    deg = np.bincount(dst, minlength=n_nodes)

    # nodes -> cores, balancing in-edge load
    order = np.argsort(-deg, kind="stable")
    cap_nodes = n_nodes // NCORES
    core_of = np.empty(n_nodes, np.int64)
    loads = np.zeros(NCORES, np.int64)
    counts = np.zeros(NCORES, np.int64)
    for n in order:
        avail = np.flatnonzero(counts < cap_nodes)
        c = avail[np.argmin(loads[avail])]
        core_of[n] = c
        loads[c] += deg[n]
        counts[c] += 1

    # per core: nodes -> blocks, balancing block load
    new_id = np.full(n_nodes, -1, np.int64)
    for c in range(NCORES):
        nodes = np.flatnonzero(core_of == c)
        nodes = nodes[np.argsort(-deg[nodes], kind="stable")]
        bload = np.zeros(nblk, np.int64)
        bcount = np.zeros(nblk, np.int64)
        slot_ctr = np.zeros(nblk, np.int64)
        for n in nodes:
            avail = np.flatnonzero(bcount < P)
            b = avail[np.argmin(bload[avail])]
            bload[b] += deg[n]
            bcount[b] += 1
            new_id[n] = c * nloc + b * P + slot_ctr[b]
            slot_ctr[b] += 1
        if bload.max() > T * P:
            raise RuntimeError(f"block overflow: {bload.max()} > {T*P}")

    nsrc = new_id[src]
    ndst = new_id[dst]
    ngb = NCORES * nblk

    # parity repair: per global block, even-src count <= cap_e, odd <= cap_o
    rng = np.random.default_rng(1)
    for _ in range(20000):
        gblk = ndst // P
        ec = np.bincount(gblk[(nsrc & 1) == 0], minlength=ngb)
        tc = np.bincount(gblk, minlength=ngb)
        bad = np.flatnonzero((ec > cap_e) | ((tc - ec) > cap_o))
        if not len(bad):
            break
        b = bad[0]
        par = 0 if ec[b] > cap_e else 1
        eidx = np.flatnonzero(gblk == b)
        cand = eidx[(nsrc[eidx] & 1) == par]
        sn = nsrc[cand[rng.integers(len(cand))]]
        sb = (sn // P) * P
        partners = np.arange(sb + (1 - par), sb + P, 2)
        pn = partners[rng.integers(len(partners))]
        for arr in (nsrc, ndst):
            ms, mp = arr == sn, arr == pn
            arr[ms] = pn
            arr[mp] = sn
        os_ = np.flatnonzero(new_id == sn)[0]
        op_ = np.flatnonzero(new_id == pn)[0]
        new_id[os_], new_id[op_] = pn, sn
    else:
        raise RuntimeError("parity repair failed")

    # per-core streams in tile order:
    # group-major; within a group of G blocks: [all even chunks | all odd chunks]
    plans = []
    for c in range(NCORES):
        sel = (ndst >= c * nloc) & (ndst < (c + 1) * nloc)
        es, ed = nsrc[sel], ndst[sel] - c * nloc
        b = ed // P
        par = (es & 1).astype(np.int64)
        o = np.lexsort((es, par, b))
        es, ed, b, par = es[o], ed[o], b[o], par[o]
        # per-block, per-parity slot ranges (block-major natural order first)
        idx_e = np.full((nblk, cap_e), -1, np.int64)
        ds_e = np.zeros((nblk, cap_e), np.int64)
        idx_o = np.full((nblk, cap_o), -1, np.int64)
        ds_o = np.zeros((nblk, cap_o), np.int64)
        for blk in range(nblk):
            m = b == blk
            for pp, idx_a, ds_a, cap in ((0, idx_e, ds_e, cap_e), (1, idx_o, ds_o, cap_o)):
                mm = m & (par == pp)
                k = int(mm.sum())
                assert k <= cap
                idx_a[blk, :k] = es[mm]
                ds_a[blk, :k] = ed[mm] % P
        plans.append(dict(idx_e=idx_e, ds_e=ds_e, idx_o=idx_o, ds_o=ds_o))
    return new_id, plans


def _wrap_idx(a):
    """[k] int -> wrapped [128, k/16] int16 (16-partition wrap, replicated x8)."""
    w = a.reshape(-1, 16).T.astype(np.int16)
    return np.ascontiguousarray(np.tile(w, (8, 1)))


# ----------------------------------------------------------------------------
# Bass kernel builder
# ----------------------------------------------------------------------------

def build_kernel(cfg):
    """cfg: dict(nblk, te, to, g, n_nodes). Returns nc."""
    nblk, te, to, G = cfg["nblk"], cfg["te"], cfg["to"], cfg["g"]
    skip_ag = cfg.get("skip_ag", False)
    skip_gather = cfg.get("skip_gather", False)
    T = te + to
    nloc = nblk * P
    npair_c = nloc // 2              # pairs per core (incl dummies)
    npair = NCORES * npair_c         # rows per parity table (excl sentinel)
    groups = [(i, min(i + G, nblk)) for i in range(0, nblk, G)]

    ROW1 = HEADS * (HID + 1)                 # 260 bf16 cols of payload
    TBLW = 384                                # L0/L1 row width (bf16 cols)
    TBLW2 = 128                               # L2 row width
    SOFF = 260                                # s_dst f32 at bf16 col 260 (byte 520)
    SOFF2 = 66                                # L2: s at col 66 (byte 132)
    NCH = [ROW1, ROW1, HID + 1]               # matmul rhs widths per layer
    ROWW = [TBLW, TBLW, TBLW2]
    SOFFS = [SOFF, SOFF, SOFF2]
    NH = [HEADS, HEADS, 1]

    nc = bacc.Bacc("TRN2", num_devices=NCORES)
    dt = nc.dram_tensor

    # ---- inputs
    x_in = dt("x_slice", [nloc, F_IN], F32, kind="ExternalInput")
    W_in = [dt(f"W{l}", [F_IN if l == 0 else HEADS * HID, (HEADS if l < 2 else 1) * HID], F32, kind="ExternalInput") for l in range(3)]
    Wc_in = dt("Wc", [HID, OUTD], F32, kind="ExternalInput")
    A_in = [dt(f"A{l}", [(HEADS if l < 2 else 1) * HID, 2 * (HEADS if l < 2 else 1)], F32, kind="ExternalInput") for l in range(3)]
    bn_in = [dt(f"bn{l}", [5, (HEADS if l < 2 else 1) * HID], F32, kind="ExternalInput") for l in range(3)]  # rows: b,g,bt,m,v
    bc_in = dt("bc", [1, OUTD], F32, kind="ExternalInput")
    ident_in = dt("ident", [P, P], F32, kind="ExternalInput")
    iota_in = dt("iota", [P, P], BF16, kind="ExternalInput")

    n_e = te * P
    n_o = to * P
    ntot_ch = sum((g1 - g0) * T for g0, g1 in groups)
    idx_e_in = dt("idx_e", [P, nblk * n_e // 16], I16, kind="ExternalInput")
    idx_o_in = dt("idx_o", [P, nblk * n_o // 16], I16, kind="ExternalInput")
    idx_sd_in = dt("idx_sd", [P, nblk * T * P // 16], I16, kind="ExternalInput")
    dslot_in = dt("dslot", [P, nblk * T], BF16, kind="ExternalInput")
    dpar_in = dt("dpar", [P, nblk * T], F32, kind="ExternalInput")

    y_out = dt("y", [nloc, OUTD], F32, kind="ExternalOutput")

    # ---- internal DRAM: tables + staging
    tab_e = [dt(f"tab{l}e", [npair + 1, ROWW[l]], BF16, kind="Internal", addr_space="Shared") for l in range(3)]
    tab_o = [dt(f"tab{l}o", [npair + 1, ROWW[l]], BF16, kind="Internal", addr_space="Shared") for l in range(3)]
    tab_s = [dt(f"tab{l}s", [npair + 1, 64], F32, kind="Internal", addr_space="Shared") for l in range(3)]
    stg_e = [dt(f"stg{l}e", [npair_c, ROWW[l]], BF16, kind="Internal") for l in range(3)]
    stg_o = [dt(f"stg{l}o", [npair_c, ROWW[l]], BF16, kind="Internal") for l in range(3)]
    stg_s = [dt(f"stg{l}s", [npair_c, 64], F32, kind="Internal") for l in range(3)]

    rg = [list(range(NCORES))]

    with tile.TileContext(nc) as tc, ExitStack() as ctx:
        cst = ctx.enter_context(tc.tile_pool(name="cst", bufs=1))
        wrk = ctx.enter_context(tc.tile_pool(name="wrk", bufs=2))
        pk = ctx.enter_context(tc.tile_pool(name="pk", bufs=2))
        ps = ctx.enter_context(tc.tile_pool(name="ps", bufs=2, space="PSUM"))
        ps1 = ctx.enter_context(tc.tile_pool(name="ps1", bufs=1, space="PSUM"))

        # ---- constants to SBUF
        ident = cst.tile([P, P], F32)
        nc.sync.dma_start(ident[:], ident_in[:])
        ones1 = cst.tile([1, P], BF16)
        nc.vector.memset(ones1[:], 1.0)

        def bcast_row(dst_sb, row_ap, w):
            # dst_sb[p, :w] = row_ap[0, :w] for all partitions, via K=1 matmul
            bp = ps1.tile([P, 512], F32, tag="bcps")
            rb = wrk.tile([1, 512], BF16, tag="rbf")
            nc.vector.tensor_copy(rb[:, :w], row_ap)
            nc.tensor.matmul(bp[:, :w], ones1[:], rb[:, :w], start=True, stop=True)
            nc.vector.tensor_copy(dst_sb, bp[:, :w])
        iota = cst.tile([P, P], BF16)
        nc.sync.dma_start(iota[:], iota_in[:])
        dslot_sb = cst.tile([P, nblk * T], BF16)
        nc.sync.dma_start(dslot_sb[:], dslot_in[:])
        dpar_sb = cst.tile([P, nblk * T], F32)
        nc.sync.dma_start(dpar_sb[:], dpar_in[:])
        bc_rep = cst.tile([P, OUTD], F32)
        bcl = wrk.tile([1, OUTD], F32, tag="bcl")
        nc.sync.dma_start(bcl[:], bc_in[:])
        bcast_row(bc_rep[:], bcl[0:1, :], OUTD)

        # per-layer weight tiles (bf16) + WA + BN affine
        Wt = []     # list of [K-chunk bf16 tiles]  (rhs for dense)
        WAt = []    # [K-chunk [128, 2*nh] bf16]
        Wct = None
        sc_rep = []
        sh_rep = []
        for l in range(3):
            fin = F_IN if l == 0 else HEADS * HID
            fout = (HEADS if l < 2 else 1) * HID * (HEADS if False else 1)
            fout = (HEADS if l < 2 else 1) * HID
            nkin = fin // P
            # load W f32, cast to bf16 chunk tiles
            wf = wrk.tile([P, (fin // P) * fout], F32, tag="wload")
            nc.sync.dma_start(wf[:].rearrange("p (k f) -> p k f", k=nkin),
                              W_in[l].rearrange("(k p) f -> p k f", p=P))
            wb = cst.tile([P, (fin // P) * fout], BF16, tag=f"wb{l}")
            nc.vector.tensor_copy(wb[:], wf[:])
            Wt.append([wb[:, k * fout:(k + 1) * fout] for k in range(nkin)])

            # WT (for WA): transpose W chunks: W [fin, fout] -> WT [fout, fin]
            nko = fout // P if fout >= P else 1
            fo_p = min(fout, P)
            wtb = cst.tile([P, (fout // P if fout >= P else 1) * 0 + nkin * ((fout + P - 1) // P) * 0 + nkin * P], BF16, tag=f"wt{l}") if False else None
            # simpler: WT stored as f32 sbuf [fout rows over ceil chunks][fin]
            nchT = (fout + P - 1) // P
            wT = cst.tile([P, nchT * fin], F32, tag=f"wT{l}")
            for ki in range(nkin):          # W row chunk (fin dim)
                for kj in range(nchT):      # W col chunk (fout dim)
                    cw = min(P, fout - kj * P)
                    tp = ps1.tile([P, P], F32, tag="tph")
                    nc.tensor.transpose(tp[:cw, :P], wf[:, ki * fout + kj * P: ki * fout + kj * P + cw], ident[:])
                    nc.vector.tensor_copy(wT[:cw, kj * fin + ki * P: kj * fin + (ki + 1) * P], tp[:cw, :P])
            # A -> bf16
            nh = NH[l]
            fo_p = min(P, fout)
            af = wrk.tile([P, nchT * 2 * nh], F32, tag="aload")
            if fout >= P:
                nc.sync.dma_start(af[:].rearrange("p (k f) -> p k f", k=nchT),
                                  A_in[l].rearrange("(k p) f -> p k f", p=P))
            else:
                nc.sync.dma_start(af[:fo_p, 0:2 * nh], A_in[l][:])
            ab = af
            # WA [fin, 2nh] = sum_kj WT_chunk.T @ A_chunk
            wab = cst.tile([P, nkin * 2 * nh], F32, tag=f"wab{l}")
            for ki in range(nkin):
                wa_ps = ps1.tile([P, 2 * nh], F32, tag="sps")
                for kj in range(nchT):
                    cw = min(P, fout - kj * P)
                    nc.tensor.matmul(wa_ps[:],
                                     wT[:cw, kj * fin + ki * P: kj * fin + (ki + 1) * P],
                                     ab[:cw, kj * 2 * nh:(kj + 1) * 2 * nh],
                                     start=(kj == 0), stop=(kj == nchT - 1))
                nc.vector.tensor_copy(wab[:, ki * 2 * nh:(ki + 1) * 2 * nh], wa_ps[:])
            WAt.append([wab[:, k * 2 * nh:(k + 1) * 2 * nh] for k in range(nkin)])

            # BN affine: scale2 = g/sqrt(v+eps); shift2 = (b - m)*scale2 + bt
            bn = wrk.tile([1, 5 * fout], F32, tag="bnload")
            nc.sync.dma_start(bn[:].rearrange("p (r f) -> p r f", r=5), bn_in[l][None, :, :])
            bnr = [bn[:, i * fout:(i + 1) * fout] for i in range(5)]  # b,g,bt,m,v
            sc1 = wrk.tile([1, fout], F32, tag="sc1")
            nc.vector.tensor_scalar(out=sc1[:], in0=bnr[4], scalar1=EPS_BN, scalar2=None, op0=mybir.AluOpType.add)
            nc.scalar.activation(sc1[:], sc1[:], mybir.ActivationFunctionType.Sqrt)
            rc = wrk.tile([1, fout], F32, tag="rc1")
            nc.vector.reciprocal(rc[:], sc1[:])
            nc.vector.tensor_tensor(out=rc[:], in0=rc[:], in1=bnr[1], op=mybir.AluOpType.mult)
            sh1 = wrk.tile([1, fout], F32, tag="sh1")
            nc.vector.tensor_tensor(out=sh1[:], in0=bnr[0], in1=bnr[3], op=mybir.AluOpType.subtract)
            nc.vector.tensor_tensor(out=sh1[:], in0=sh1[:], in1=rc[:], op=mybir.AluOpType.mult)
            nc.vector.tensor_tensor(out=sh1[:], in0=sh1[:], in1=bnr[2], op=mybir.AluOpType.add)
            screp = cst.tile([P, fout], F32, tag=f"screp{l}")
            bcast_row(screp[:], rc[0:1, :], fout)
            shrep = cst.tile([P, fout], F32, tag=f"shrep{l}")
            bcast_row(shrep[:], sh1[0:1, :], fout)
            sc_rep.append(screp)
            sh_rep.append(shrep)

        wcf = wrk.tile([HID, OUTD], F32, tag="wcl")
        nc.sync.dma_start(wcf[:], Wc_in[:])
        Wct = cst.tile([HID, OUTD], BF16)
        nc.vector.tensor_copy(Wct[:], wcf[:])

        # ---- sentinel rows: feats 0 (incl ones col), s slots NEG
        sent = cst.tile([1, TBLW], BF16)
        nc.vector.memset(sent[:], 0)
        sent32 = sent[:].bitcast(F32)
        nc.vector.memset(sent32[:, SOFF // 2: SOFF // 2 + 8], NEG)
        sent2 = cst.tile([1, TBLW2], BF16)
        nc.vector.memset(sent2[:], 0)
        nc.vector.memset(sent2[:].bitcast(F32)[:, SOFF2 // 2: SOFF2 // 2 + 2], NEG)
        sentS = cst.tile([1, 64], F32)
        nc.vector.memset(sentS[:], NEG)
        for l in range(3):
            st = sent2 if l == 2 else sent
            nc.sync.dma_start(tab_e[l][npair:npair + 1, :], st[:])
            nc.sync.dma_start(tab_o[l][npair:npair + 1, :], st[:])
            nc.sync.dma_start(tab_s[l][npair:npair + 1, :], sentS[:])

        # ================= per-layer finalize helper ====================
        def finalize_block(l, zsrc_ps, b_in_grp, gi, pack, pack_s, pack_y):
            """zsrc_ps: agg psum [P, NCH[l]] (cols: per-head 64 feats + denom).
            Produces next-layer packed rows into pack/pack_s (this group's
            staging tiles), or y into pack_y for l==2."""
            nh = NH[l]
            fout = nh * HID
            # denominators -> reciprocal
            dn = wrk.tile([P, nh], F32, tag="dn")
            dcols = zsrc_ps[:].rearrange("p (h c) -> p h c", c=HID + 1)[:, :, HID]
            nc.vector.tensor_scalar(out=dn[:], in0=dcols, scalar1=1e-30, scalar2=None, op0=mybir.AluOpType.add)
            rd = wrk.tile([P, nh], F32, tag="rd")
            nc.vector.reciprocal(rd[:], dn[:])
            # normalize + affine + activation -> h tile [P, fout] f32
            h = wrk.tile([P, fout], F32, tag="h")
            for hd in range(nh):
                nc.vector.tensor_scalar(
                    out=h[:, hd * HID:(hd + 1) * HID],
                    in0=zsrc_ps[:, hd * (HID + 1):hd * (HID + 1) + HID],
                    scalar1=rd[:, hd, None], scalar2=None, op0=mybir.AluOpType.mult)
            nc.vector.tensor_tensor(out=h[:], in0=h[:], in1=sc_rep[l][:, :fout], op=mybir.AluOpType.mult)
            nc.vector.tensor_tensor(out=h[:], in0=h[:], in1=sh_rep[l][:, :fout], op=mybir.AluOpType.add)
            if l < 2:
                u = wrk.tile([P, fout], F32, tag="elu_u")
                nc.vector.tensor_scalar(out=u[:], in0=h[:], scalar1=0.0, scalar2=None, op0=mybir.AluOpType.min)
                nc.scalar.activation(u[:], u[:], mybir.ActivationFunctionType.Exp)
                nc.vector.tensor_scalar(out=h[:], in0=h[:], scalar1=0.0, scalar2=-1.0, op0=mybir.AluOpType.max, op1=mybir.AluOpType.add)
                nc.vector.tensor_tensor(out=h[:], in0=h[:], in1=u[:], op=mybir.AluOpType.add)
            # transpose h -> hT bf16 chunks
            nkin = fout // P if fout >= P else 1
            fo_p = min(fout, P)
            hT = wrk.tile([P, nkin * P], BF16, tag="hT")
            hTf = wrk.tile([P, nkin * P], F32, tag="hTf")
            for k in range(nkin):
                cw = min(P, fout - k * P)
                tp = ps1.tile([P, P], F32, tag="tph")
                nc.tensor.transpose(tp[:cw, :], h[:, k * P:k * P + cw], ident[:])
                nc.vector.tensor_copy(hT[:cw, k * P:(k + 1) * P], tp[:cw, :])
                nc.vector.tensor_copy(hTf[:cw, k * P:(k + 1) * P], tp[:cw, :])
            if l == 2:
                # head: y = h2 @ Wc + bc
                yp = ps1.tile([P, OUTD], F32, tag="sps")
                nc.tensor.matmul(yp[:], hT[:HID, 0:P], Wct[:], start=True, stop=True)
                nc.vector.tensor_tensor(out=pack_y[:, b_in_grp, :], in0=yp[:], in1=bc_rep[:], op=mybir.AluOpType.add)
                return
            # dense: hf_next = h @ W_{l+1}; s = h @ WA_{l+1}
            nl = l + 1
            fon = NH[nl] * HID
            hf = ps.tile([P, fon], F32, tag="hfps")
            sps = ps1.tile([P, 2 * NH[nl]], F32, tag="sps")
            for k in range(nkin):
                nc.tensor.matmul(hf[:], hT[:, k * P:(k + 1) * P], Wt[nl][k], start=(k == 0), stop=(k == nkin - 1))
            for k in range(nkin):
                nc.tensor.matmul(sps[:], hTf[:, k * P:(k + 1) * P], WAt[nl][k], start=(k == 0), stop=(k == nkin - 1))
            # pack rows: [hf_hd | 1.0]*nh + s_dst f32 + s_src f32
            nhn = NH[nl]
            nc.vector.tensor_copy(
                pack[:, b_in_grp, 0:nhn * (HID + 1)].rearrange("p (h c) -> p h c", h=nhn)[:, :, 0:HID],
                hf[:].rearrange("p (h c) -> p h c", h=nhn))
            pk32 = pack[:].bitcast(F32)
            so = SOFFS[nl] // 2
            # sps cols: [s_src (nhn) | s_dst (nhn)] -> store s_dst then s_src
            nc.vector.tensor_copy(pk32[:, b_in_grp, so:so + nhn], sps[:, nhn:2 * nhn])
            nc.vector.tensor_copy(pk32[:, b_in_grp, so + nhn:so + 2 * nhn], sps[:, 0:nhn])
            nc.vector.tensor_copy(pack_s[:, b_in_grp, :], sps[:, nhn:2 * nhn])

        # ================= L0 prep: x -> table0 rows =====================
        for g0, g1 in groups:
            gn = g1 - g0
            pack = pk.tile([P, G, TBLW], BF16, tag="pack0")
            pack_s = pk.tile([P, G, HEADS], F32, tag="packs0")
            nc.vector.memset(pack[:, :gn, :], 0)
            for b in range(g0, g1):
                bi = b - g0
                xt = wrk.tile([P, F_IN], F32, tag="xt")
                nc.sync.dma_start(xt[:], x_in[b * P:(b + 1) * P, :])
                xT = wrk.tile([P, F_IN], BF16, tag="xT")
                xTf = wrk.tile([P, F_IN], F32, tag="xTf")
                tp = ps1.tile([P, P], F32, tag="tph")
                nc.tensor.transpose(tp[:], xt[:], ident[:])
                nc.vector.tensor_copy(xT[:], tp[:])
                nc.vector.tensor_copy(xTf[:], tp[:])
                hf = ps.tile([P, HEADS * HID], F32, tag="hfps")
                sps = ps1.tile([P, 2 * HEADS], F32, tag="sps")
                nc.tensor.matmul(hf[:], xT[:], Wt[0][0], start=True, stop=True)
                nc.tensor.matmul(sps[:], xTf[:], WAt[0][0], start=True, stop=True)
                nc.vector.tensor_copy(
                    pack[:, bi, 0:HEADS * (HID + 1)].rearrange("p (h c) -> p h c", h=HEADS)[:, :, 0:HID],
                    hf[:].rearrange("p (h c) -> p h c", h=HEADS))
                pk32 = pack[:].bitcast(F32)
                so = SOFF // 2
                nc.vector.tensor_copy(pk32[:, bi, so:so + HEADS], sps[:, HEADS:2 * HEADS])
                nc.vector.tensor_copy(pk32[:, bi, so + HEADS:so + 2 * HEADS], sps[:, 0:HEADS])
                nc.vector.tensor_copy(pack_s[:, bi, :], sps[:, HEADS:2 * HEADS])
            # ones cols
            on = pack[:, 0:gn, 0:HEADS * (HID + 1)].rearrange("p g (h c) -> p g h c", h=HEADS)[:, :, :, HID:HID + 1]
            nc.vector.memset(on, 1.0)
            # staging DMAs (even/odd partitions -> pair rows)
            for par, stg in ((0, stg_e[0]), (1, stg_o[0])):
                nc.sync.dma_start(
                    stg[g0 * 64:(g0 + gn) * 64, :].rearrange("(g q) w -> q g w", g=gn),
                    pack[par::2, 0:gn, :])
            nc.sync.dma_start(
                stg_s[0][g0 * 64:(g0 + gn) * 64, 0:HEADS].rearrange("(g q) w -> q g w", g=gn),
                pack_s[0::2, 0:gn, :])
            nc.sync.dma_start(
                stg_s[0][g0 * 64:(g0 + gn) * 64, HEADS:2 * HEADS].rearrange("(g q) w -> q g w", g=gn),
                pack_s[1::2, 0:gn, :])

        # ================= layers =====================
        for l in range(3):
            nh = NH[l]
            roww = ROWW[l]
            nch_w = NCH[l]
            soff = SOFFS[l]
            # AllGather tables
            if skip_ag:
                nc.sync.dma_start(tab_e[l][0:npair_c, :], stg_e[l][:])
                nc.sync.dma_start(tab_o[l][0:npair_c, :], stg_o[l][:])
                nc.sync.dma_start(tab_s[l][0:npair_c, :], stg_s[l][:])
            else:
                nc.gpsimd.collective_compute("AllGather", mybir.AluOpType.bypass,
                                             ins=[stg_e[l][:]], outs=[tab_e[l][0:npair, :]], replica_groups=rg)
                nc.gpsimd.collective_compute("AllGather", mybir.AluOpType.bypass,
                                             ins=[stg_o[l][:]], outs=[tab_o[l][0:npair, :]], replica_groups=rg)
                nc.gpsimd.collective_compute("AllGather", mybir.AluOpType.bypass,
                                             ins=[stg_s[l][:]], outs=[tab_s[l][0:npair, :]], replica_groups=rg)

            for gi, (g0, g1) in enumerate(groups):
                gn = g1 - g0
                nche = gn * te       # even chunks in group
                ncho = gn * to
                ncht = nche + ncho
                # chunk order in tiles: [even chunks | odd chunks]
                # idx loads
                ixe = wrk.tile([P, nche * 8], I16, tag="ixe")
                nc.sync.dma_start(ixe[:], idx_e_in[:, g0 * n_e // 16:(g0 * n_e + nche * P) // 16])
                ixo = wrk.tile([P, ncho * 8], I16, tag="ixo")
                nc.sync.dma_start(ixo[:], idx_o_in[:, g0 * n_o // 16:(g0 * n_o + ncho * P) // 16])
                ixs = wrk.tile([P, ncht * 8], I16, tag="ixs")
                nc.sync.dma_start(ixs[:], idx_sd_in[:, g0 * T * 8:(g0 * T + ncht) * 8])

                g = wrk.tile([P, G * T, roww], BF16, tag="g")
                sd = wrk.tile([P, G * T, 64], F32, tag="sd")
                if skip_gather:
                    nc.vector.memset(g[:, 0:ncht, :], 0)
                    nc.vector.memset(sd[:, 0:ncht, :], 1.0)
                else:
                    nc.gpsimd.dma_gather(g[:, 0:nche, :], tab_e[l][:], ixe[:], nche * P, nche * P, roww, single_packet=False)
                    nc.gpsimd.dma_gather(g[:, nche:ncht, :], tab_o[l][:], ixo[:], ncho * P, ncho * P, roww, single_packet=False)
                    nc.gpsimd.dma_gather(sd[:, 0:ncht, :], tab_s[l][:], ixs[:], ncht * P, ncht * P, 64, single_packet=False)

                # per-edge scalars (f32): s_dst select + s_src + lrelu + exp
                g32 = g[:].bitcast(F32)
                ssrc = g32[:, 0:ncht, soff // 2 + nh: soff // 2 + 2 * nh]
                e_t = wrk.tile([P, G * T, nh], F32, tag="e_t")
                # sd cols: [even-node s_dst (nh) | odd-node s_dst (nh)]
                nc.vector.tensor_tensor(out=e_t[:, 0:ncht, :], in0=sd[:, 0:ncht, nh:2 * nh], in1=sd[:, 0:ncht, 0:nh], op=mybir.AluOpType.subtract)
                dp = dpar_sb[:, (g0 * T):(g0 * T) + ncht, None].to_broadcast([P, ncht, nh])
                nc.vector.tensor_tensor(out=e_t[:, 0:ncht, :], in0=e_t[:, 0:ncht, :], in1=dp, op=mybir.AluOpType.mult)
                nc.vector.tensor_tensor(out=e_t[:, 0:ncht, :], in0=e_t[:, 0:ncht, :], in1=sd[:, 0:ncht, 0:nh], op=mybir.AluOpType.add)
                nc.vector.tensor_tensor(out=e_t[:, 0:ncht, :], in0=e_t[:, 0:ncht, :], in1=ssrc, op=mybir.AluOpType.add)
                lr = wrk.tile([P, G * T, nh], F32, tag="lr")
                nc.vector.tensor_scalar(out=lr[:, 0:ncht, :], in0=e_t[:, 0:ncht, :], scalar1=0.2, scalar2=None, op0=mybir.AluOpType.mult)
                nc.vector.tensor_tensor(out=lr[:, 0:ncht, :], in0=lr[:, 0:ncht, :], in1=e_t[:, 0:ncht, :], op=mybir.AluOpType.max)
                ee = wrk.tile([P, G * T, nh], BF16, tag="ee")
                nc.scalar.activation(ee[:, 0:ncht, :], lr[:, 0:ncht, :], mybir.ActivationFunctionType.Exp)

                # scale gathered features by per-head ee (in place)
                for hd in range(nh):
                    nc.vector.tensor_tensor(
                        out=g[:, 0:ncht, hd * (HID + 1):(hd + 1) * (HID + 1)],
                        in0=g[:, 0:ncht, hd * (HID + 1):(hd + 1) * (HID + 1)],
                        in1=ee[:, 0:ncht, hd, None].to_broadcast([P, ncht, HID + 1]),
                        op=mybir.AluOpType.mult)

                # one-hot M
                m = wrk.tile([P, G * T, P], BF16, tag="m")
                nc.vector.tensor_tensor(
                    out=m[:, 0:ncht, :],
                    in0=iota[:, None, :].to_broadcast([P, ncht, P]),
                    in1=dslot_sb[:, (g0 * T):(g0 * T) + ncht, None].to_broadcast([P, ncht, P]),
                    op=mybir.AluOpType.is_equal)

                # aggregate per block
                pack = pk.tile([P, G, TBLW if l < 2 else TBLW], BF16, tag="packL")
                pack_s = pk.tile([P, G, max(NH[min(l + 1, 2)], 1)], F32, tag="packsL")
                if l == 2:
                    pack_y = pk.tile([P, G, OUTD], F32, tag="packy")
                else:
                    pack_y = None
                for b in range(g0, g1):
                    bi = b - g0
                    acc = ps.tile([P, nch_w], F32, tag="agg")
                    chunks = [bi * te + j for j in range(te)] + [nche + bi * to + j for j in range(to)]
                    for ci, ch in enumerate(chunks):
                        nc.tensor.matmul(acc[:], m[:, ch, :], g[:, ch, 0:nch_w],
                                         start=(ci == 0), stop=(ci == len(chunks) - 1))
                    finalize_block(l, acc, bi, gi, pack, pack_s, pack_y)
                if l < 2:
                    nl = l + 1
                    nhn = NH[nl]
                    on = pack[:, 0:gn, 0:nhn * (HID + 1)].rearrange("p g (h c) -> p g h c", h=nhn)[:, :, :, HID:HID + 1]
                    nc.vector.memset(on, 1.0)
                    for par, stg in ((0, stg_e[nl]), (1, stg_o[nl])):
                        nc.sync.dma_start(
                            stg[g0 * 64:(g0 + gn) * 64, :].rearrange("(g q) w -> q g w", g=gn),
                            pack[par::2, 0:gn, 0:ROWW[nl]])
                    nc.sync.dma_start(
                        stg_s[nl][g0 * 64:(g0 + gn) * 64, 0:nhn].rearrange("(g q) w -> q g w", g=gn),
                        pack_s[0::2, 0:gn, :])
                    nc.sync.dma_start(
                        stg_s[nl][g0 * 64:(g0 + gn) * 64, nhn:2 * nhn].rearrange("(g q) w -> q g w", g=gn),
                        pack_s[1::2, 0:gn, :])
                else:
                    nc.sync.dma_start(
                        y_out[g0 * P:g1 * P, :].rearrange("(g q) w -> q g w", g=gn),
                        pack_y[:, 0:gn, :])

    nc.compile()
    return nc


# ----------------------------------------------------------------------------
# Host wrapper
# ----------------------------------------------------------------------------

_CACHE = {}


def _scatter_A(a_s, a_d):
    nh, hc = a_s.shape
    A = np.zeros((nh * hc, 2 * nh), np.float32)
    for hd in range(nh):
        A[hd * hc:(hd + 1) * hc, hd] = a_s[hd]
        A[hd * hc:(hd + 1) * hc, nh + hd] = a_d[hd]
    return A


def run(inputs, nblk, te, to, G, n_nodes, trace=False):
    T = te + to
    nloc = nblk * P
    npair = NCORES * nloc // 2
    groups = [(i, min(i + G, nblk)) for i in range(0, nblk, G)]
    edge_index = np.asarray(inputs["edge_index"])
    x = np.asarray(inputs["x"], np.float32)

    new_id, plans = preprocess(edge_index, n_nodes, nblk, te, to)

    import os
    skip_ag = bool(int(os.environ.get("K_SKIP_AG", "0")))
    skip_gather = bool(int(os.environ.get("K_SKIP_GATHER", "0")))
    key = (nblk, te, to, G, skip_ag, skip_gather)
    if key not in _CACHE:
        _CACHE[key] = build_kernel(dict(nblk=nblk, te=te, to=to, g=G, n_nodes=n_nodes,
                                        skip_ag=skip_ag, skip_gather=skip_gather))
    nc = _CACHE[key]

    # common inputs
    iota = np.tile(np.arange(P, dtype=BF), (P, 1))
    ident = np.eye(P, dtype=np.float32)
    Wl = {f"W{l}": np.asarray(inputs[f"W{l}"], np.float32) for l in range(3)}
    Al = {f"A{l}": _scatter_A(np.asarray(inputs[f"as{l}"], np.float32),
                              np.asarray(inputs[f"ad{l}"], np.float32)) for l in range(3)}
    bnl = {f"bn{l}": np.stack([np.asarray(inputs[k + str(l)], np.float32)
                               for k in ("b", "g", "bt", "m", "v")]) for l in range(3)}

    in_maps = []
    for c in range(NCORES):
        pl = plans[c]
        # x slice (new order); dummies zero
        xs = np.zeros((nloc, F_IN), np.float32)
        ids = new_id - c * nloc
        mine = (ids >= 0) & (ids < nloc)
        xs[ids[mine]] = x[mine]
        # streams in tile order
        fe_l, fo_l, sd_l, dsl_l, dpr_l = [], [], [], [], []
        for g0, g1 in groups:
            blks = list(range(g0, g1))
            ie = np.concatenate([pl["idx_e"][b] for b in blks])
            io = np.concatenate([pl["idx_o"][b] for b in blks])
            de = np.concatenate([pl["ds_e"][b] for b in blks])
            do = np.concatenate([pl["ds_o"][b] for b in blks])
            fe_l.append(np.where(ie >= 0, ie >> 1, npair))
            fo_l.append(np.where(io >= 0, io >> 1, npair))
            # s_dst pair rows (global dst new id >> 1); sentinel for pads
            dg_e = np.concatenate([(c * nloc + b * P + pl["ds_e"][b]) for b in blks])
            dg_o = np.concatenate([(c * nloc + b * P + pl["ds_o"][b]) for b in blks])
            valid = np.concatenate([ie, io]) >= 0
            dg = np.concatenate([dg_e, dg_o])
            sd_l.append(np.where(valid, dg >> 1, npair))
            dsl_l.append(np.concatenate([de, do]))
            dpr_l.append(np.concatenate([dg_e & 1, dg_o & 1]).astype(np.float32) * valid)
        idx_e = _wrap_idx(np.concatenate(fe_l))
        idx_o = _wrap_idx(np.concatenate(fo_l))
        idx_sd = _wrap_idx(np.concatenate(sd_l))
        dslot = np.ascontiguousarray(
            np.concatenate(dsl_l).reshape(-1, P).T.astype(BF))
        dpar = np.ascontiguousarray(
            np.concatenate(dpr_l).reshape(-1, P).T.astype(np.float32))
        im = dict(x_slice=xs, idx_e=idx_e, idx_o=idx_o, idx_sd=idx_sd,
                  dslot=dslot, dpar=dpar,
                  Wc=np.asarray(inputs["Wc"], np.float32),
                  bc=np.asarray(inputs["bc"], np.float32).reshape(1, OUTD),
                  ident=ident, iota=iota)
        im.update(Wl)
        im.update(Al)
        im.update(bnl)
        in_maps.append(im)

    import time
    res = run_bass_kernel_spmd(nc, in_maps, core_ids=list(range(NCORES)), trace=False)
    if trace:
        # warm timing runs: wall-clock of the PJRT execute path
        walls = []
        for _ in range(3):
            t0 = time.perf_counter()
            res = run_bass_kernel_spmd(nc, in_maps, core_ids=list(range(NCORES)), trace=False)
            walls.append(time.perf_counter() - t0)
        res.exec_time_ns = int(min(walls) * 1e9)
    y_cat = np.concatenate([r["y"] for r in res.results])  # [NCORES*nloc, 2]
    return y_cat[new_id], res


def kernel(**inputs) -> np.ndarray:
    out, _ = run(inputs, nblk=49, te=9, to=9, G=3, n_nodes=N_FULL)
    return out.astype(np.float32)

